# revision 8
# baseline (speedup 1.0000x reference)
"""GATConv x2 + LayerNorm (GNN message passing) on 8 TRN2 NeuronCores.

v2 strategy (edge-parallel, dst-sharded, batched gathers):
  - Nodes sharded across 8 cores by id range; windows of 128 dst slots with
    in-degree balancing (host greedy).  Edges laid out per window in
    128-edge blocks, split lo/hi by source table row (int16 idx limit).
  - Phase 0 (replicated): every core computes the FULL layer-1 node table
    t1 = [h1|e_src1] for all 50176 slots (x @ [W1|W1@a_src|W1@a_dst], one
    matmul per window), so no collective is needed for it.  e_dst1 goes to
    256B-row side tables (lo/hi).
  - Phase 1: per 7-window group, 2 giant dma_gathers fetch source rows;
    per window, a one-hot S matrix (dst offsets) is built on-chip, each
    128x128 block is PE-transposed to broadcast e_dst to edges via small
    matmuls, then exp(leaky_relu(e)) weights the rows and one matmul per
    block accumulates messages + softmax denominators in PSUM.  Epilogue
    normalizes, relu, transposes o1 into a [256, SL] shard.
  - One AllGather of o1T (25.7MB); every core then computes the FULL
    layer-2 table t2 = [h2|e_src2] (h2 = o1 @ W2) redundantly.
  - Phase 2: same edge machinery on t2; epilogue does head-mean, bias,
    LayerNorm, output rows (window-slot order; host unpermutes).

All gathers are batched into few large calls (per-call overhead ~0.1ms
dominates descriptor cost); all small DMAs are chunked across windows.
"""

import sys

sys.path.insert(0, "/opt/trn_rl_repo")

import math
import os
import numpy as np

import concourse.bass as bass
import concourse.bacc as bacc
import concourse.mybir as mybir
from concourse import tile
from concourse.bass_utils import run_bass_kernel_spmd

F32 = mybir.dt.float32
BF16 = mybir.dt.bfloat16
I16 = mybir.dt.int16
I32 = mybir.dt.int32
AF = mybir.ActivationFunctionType
ALU = mybir.AluOpType

CORES = 8
WIN = 128
H = 4

NEG_SLOPE = 0.2
EPS_LN = 1e-5


class Cfg:
    def __init__(self, N, IN_DIM, C1, C2, KL, KH):
        assert N % CORES == 0
        self.N = N
        self.IN = IN_DIM          # 128
        self.C1 = C1              # 64
        self.C2 = C2              # 128
        self.F1 = H * C1          # 256
        self.F2 = H * C2          # 512
        self.NODES_PC = N // CORES
        self.NW = math.ceil(self.NODES_PC / WIN)
        self.SLOTS_PC = self.NW * WIN
        self.TOT = CORES * self.SLOTS_PC
        self.HALF = self.TOT // 2
        assert self.HALF <= 32767
        assert self.SLOTS_PC + 1 <= 32767
        self.KL = KL
        self.KH = KH
        self.K = KL + KH
        self.R1 = 384             # t1 row cols (bf16): 768B elem
        self.R2 = 640             # t2 row cols (bf16): 1280B elem
        self.key = (N, IN_DIM, C1, C2, KL, KH)


def _wrap_idx(arr):
    """[NWxL] int -> [128, NW*L/16] int16 wrapped+replicated per call."""
    nw, L = arr.shape
    assert L % 16 == 0
    w = arr.reshape(nw, L // 16, 16).transpose(0, 2, 1)  # [nw, 16, L/16]
    w = np.concatenate([w] * 8, axis=1)                  # [nw, 128, L/16]
    w = np.concatenate(list(w), axis=1)                  # [128, nw*L/16]
    return np.ascontiguousarray(w.astype(np.int16))


def prep(x, edge_index, W1, a_src1, a_dst1, b1, W2, a_src2, a_dst2, b2,
         gamma, beta):
    """Host-side sharding. Returns (cfg, in_maps, slot_global)."""
    N, IN_DIM = x.shape
    C1 = a_src1.shape[1]
    C2 = a_src2.shape[1]
    F1, F2 = H * C1, H * C2

    src = np.asarray(edge_index[0], dtype=np.int64)
    dst = np.asarray(edge_index[1], dtype=np.int64)
    loop = np.arange(N, dtype=np.int64)
    src = np.concatenate([src, loop])
    dst = np.concatenate([dst, loop])

    NODES_PC = N // CORES
    NW = math.ceil(NODES_PC / WIN)
    SLOTS_PC = NW * WIN

    # ---- window assignment per core (balance in-degree across NW bins) ----
    deg = np.bincount(dst, minlength=N)
    slot_global = np.empty(N, dtype=np.int64)
    win_of = np.empty(N, dtype=np.int64)
    off_of = np.empty(N, dtype=np.int64)
    import heapq
    for c in range(CORES):
        nodes = np.arange(c * NODES_PC, (c + 1) * NODES_PC)
        d = deg[nodes]
        order = np.argsort(-d, kind="stable")
        wsel = np.empty(len(nodes), dtype=np.int64)
        osel = np.empty(len(nodes), dtype=np.int64)
        heap = [(0, 0, w) for w in range(NW)]
        heapq.heapify(heap)
        for i in order:
            while True:
                load, cnt, w = heapq.heappop(heap)
                if cnt < WIN:
                    break
            wsel[i] = w
            osel[i] = cnt
            heapq.heappush(heap, (load + d[i], cnt + 1, w))
        win_of[nodes] = wsel
        off_of[nodes] = osel
        slot_global[nodes] = c * SLOTS_PC + wsel * WIN + osel

    HALF = CORES * SLOTS_PC // 2

    # ---- per-core edge layout ----
    owner = dst // NODES_PC
    src_row = slot_global[src]
    e_w = win_of[dst]
    e_off = off_of[dst]
    e_low = src_row < HALF

    KL = 0
    KH = 0
    per_core = []
    for c in range(CORES):
        m = owner == c
        ew, eo, er, el = e_w[m], e_off[m], src_row[m], e_low[m]
        nlo = np.bincount(ew[el], minlength=NW)
        nhi = np.bincount(ew[~el], minlength=NW)
        KL = max(KL, int(np.ceil(nlo.max() / WIN)))
        KH = max(KH, int(np.ceil(nhi.max() / WIN)))
        per_core.append((ew, eo, er, el))
    cfg = Cfg(N, IN_DIM, C1, C2, KL, KH)
    K = cfg.K
    TOT = cfg.TOT

    # ---- replicated inputs ----
    # x in slot order, transposed, bf16, zero-padded to TOT slots
    import ml_dtypes
    xs_full = np.zeros((TOT, IN_DIM), dtype=np.float32)
    xs_full[slot_global] = np.asarray(x, dtype=np.float32)
    xT_bf = np.ascontiguousarray(xs_full.T.astype(ml_dtypes.bfloat16))

    W1f = np.asarray(W1, np.float32)
    va1 = np.stack([W1f[:, h * C1:(h + 1) * C1] @ np.asarray(a_src1, np.float32)[h]
                    for h in range(H)], axis=1)            # [IN, H]
    vd1 = np.stack([W1f[:, h * C1:(h + 1) * C1] @ np.asarray(a_dst1, np.float32)[h]
                    for h in range(H)], axis=1)
    W1ext = np.concatenate([W1f, va1, vd1], axis=1)        # [IN, F1+8]
    W1ext_bf = np.ascontiguousarray(W1ext.astype(ml_dtypes.bfloat16))

    W2f = np.asarray(W2, np.float32)
    va2 = np.stack([W2f[:, h * C2:(h + 1) * C2] @ np.asarray(a_src2, np.float32)[h]
                    for h in range(H)], axis=1)            # [F1, H]
    vd2 = np.stack([W2f[:, h * C2:(h + 1) * C2] @ np.asarray(a_dst2, np.float32)[h]
                    for h in range(H)], axis=1)
    vw2 = np.concatenate([va2, vd2], axis=1)               # [F1, 8]
    W2a_bf = np.ascontiguousarray(W2f[0:128].astype(ml_dtypes.bfloat16))
    W2b_bf = np.ascontiguousarray(W2f[128:256].astype(ml_dtypes.bfloat16))
    vw2a_bf = np.ascontiguousarray(vw2[0:128].astype(ml_dtypes.bfloat16))
    vw2b_bf = np.ascontiguousarray(vw2[128:256].astype(ml_dtypes.bfloat16))

    in_maps = []
    for c in range(CORES):
        ew, eo, er, el = per_core[c]
        idx_lo = np.zeros((NW, KL * WIN), dtype=np.int64)
        idx_hi = np.zeros((NW, KH * WIN), dtype=np.int64)
        dstoffT = np.full((NW, K * WIN), 999.0, dtype=np.float32)
        for w in range(NW):
            wm = ew == w
            lo_m = wm & el
            hi_m = wm & ~el
            rlo, olo = er[lo_m], eo[lo_m]
            rhi, ohi = er[hi_m] - HALF, eo[hi_m]
            idx_lo[w, :len(rlo)] = rlo
            idx_hi[w, :len(rhi)] = rhi
            dstoffT[w, :len(olo)] = olo
            dstoffT[w, KL * WIN:KL * WIN + len(ohi)] = ohi
        dT = dstoffT.reshape(NW * K, WIN).T  # [128, NW*K]

        # ed-gather idx: own slots (+1 shift; 0 -> zero row of other table)
        slots = np.arange(SLOTS_PC, dtype=np.int64)
        if c * SLOTS_PC < HALF:
            eil = c * SLOTS_PC + slots + 1
            eih = np.zeros(SLOTS_PC, dtype=np.int64)
        else:
            eil = np.zeros(SLOTS_PC, dtype=np.int64)
            eih = c * SLOTS_PC - HALF + slots + 1

        in_maps.append({
            "xT": xT_bf,
            "idx_lo": _wrap_idx(idx_lo),
            "idx_hi": _wrap_idx(idx_hi),
            "edidx_lo": _wrap_idx(eil.reshape(1, -1)),
            "edidx_hi": _wrap_idx(eih.reshape(1, -1)),
            "dstoffT": np.ascontiguousarray(dT),
            "W1ext": W1ext_bf,
            "W2a": W2a_bf, "W2b": W2b_bf,
            "vw2a": vw2a_bf, "vw2b": vw2b_bf,
            "b1": np.asarray(b1, np.float32).reshape(1, -1),
            "b2": np.asarray(b2, np.float32).reshape(1, -1),
            "gamma": np.asarray(gamma, np.float32).reshape(1, -1),
            "beta": np.asarray(beta, np.float32).reshape(1, -1),
        })
    return cfg, in_maps, slot_global


# --------------------------------------------------------------------------
# device program
# --------------------------------------------------------------------------

def build(cfg):
    PH = os.environ.get("GAT_PHASES", "01AI2")
    REPEAT = int(os.environ.get("GAT_REPEAT", "1"))
    GRP1 = int(os.environ.get("GAT_GRP1", "6"))
    GRP2 = int(os.environ.get("GAT_GRP2", "3"))
    CH0 = 7   # phase-0 / interlayer chunk (divides NW=49)
    nc = bacc.Bacc("TRN2", target_bir_lowering=False, debug=False,
                   num_devices=CORES, dynamic_dma_scratch_size=49152)
    NW, K, KL, KH = cfg.NW, cfg.K, cfg.KL, cfg.KH
    F1, F2, R1, R2 = cfg.F1, cfg.F2, cfg.R1, cfg.R2
    C1, C2 = cfg.C1, cfg.C2
    SL, TOT, HALF = cfg.SLOTS_PC, cfg.TOT, cfg.HALF
    RG = [list(range(CORES))]
    HROWS = HALF + 1

    # ---- kernel I/O ----
    xT = nc.dram_tensor("xT", [cfg.IN, TOT], BF16, kind="ExternalInput")
    idx_lo = nc.dram_tensor("idx_lo", [128, NW * KL * 8], I16, kind="ExternalInput")
    idx_hi = nc.dram_tensor("idx_hi", [128, NW * KH * 8], I16, kind="ExternalInput")
    edidx_lo = nc.dram_tensor("edidx_lo", [128, SL // 16], I16, kind="ExternalInput")
    edidx_hi = nc.dram_tensor("edidx_hi", [128, SL // 16], I16, kind="ExternalInput")
    dstoffT = nc.dram_tensor("dstoffT", [128, NW * K], F32, kind="ExternalInput")
    W1ext = nc.dram_tensor("W1ext", [cfg.IN, F1 + 8], BF16, kind="ExternalInput")
    W2a = nc.dram_tensor("W2a", [128, F2], BF16, kind="ExternalInput")
    W2b = nc.dram_tensor("W2b", [128, F2], BF16, kind="ExternalInput")
    vw2a = nc.dram_tensor("vw2a", [128, 8], BF16, kind="ExternalInput")
    vw2b = nc.dram_tensor("vw2b", [128, 8], BF16, kind="ExternalInput")
    vecs = {}
    for nm, d in [("b1", F1), ("b2", C2), ("gamma", C2), ("beta", C2)]:
        vecs[nm] = nc.dram_tensor(nm, [1, d], F32, kind="ExternalInput")
    out = nc.dram_tensor("out", [SL, C2], F32, kind="ExternalOutput")

    # ---- internal DRAM ----
    t1 = nc.dram_tensor("t1", [TOT + 1, R1], BF16, kind="Internal")
    t2 = nc.dram_tensor("t2", [TOT + 1, R2], BF16, kind="Internal")
    ed1_lo = nc.dram_tensor("ed1_lo", [HROWS, 64], F32, kind="Internal")
    ed1_hi = nc.dram_tensor("ed1_hi", [HROWS, 64], F32, kind="Internal")
    ed2_lo = nc.dram_tensor("ed2_lo", [HROWS, 64], F32, kind="Internal")
    ed2_hi = nc.dram_tensor("ed2_hi", [HROWS, 64], F32, kind="Internal")
    o1T_shard = nc.dram_tensor("o1T_shard", [2 * 128, SL], BF16, kind="Internal")
    o1T_full = nc.dram_tensor("o1T_full", [CORES * 2 * 128, SL], BF16,
                              kind="Internal", addr_space="Shared")

    with tile.TileContext(nc) as tc:
        with tc.tile_pool(name="const", bufs=1) as cp:
            # iota over K*128 cols (value = col % 128), partition iota
            iota_i = cp.tile([128, K * 128], I32)
            nc.gpsimd.iota(iota_i[:], pattern=[[0, K], [1, 128]], base=0,
                           channel_multiplier=0)
            iotaK_bf = cp.tile([128, K * 128], BF16)
            nc.vector.tensor_copy(iotaK_bf[:], iota_i[:])
            iota_f = cp.tile([128, 128], F32)
            nc.vector.tensor_copy(iota_f[:], iota_i[:, 0:128])
            ic_i = cp.tile([128, 1], I32)
            nc.gpsimd.iota(ic_i[:], pattern=[[0, 1]], base=0,
                           channel_multiplier=1)
            ic_f = cp.tile([128, 1], F32)
            nc.vector.tensor_copy(ic_f[:], ic_i[:])
            ident = cp.tile([128, 128], F32)
            nc.vector.tensor_scalar(ident[:], iota_f[:], ic_f[:, 0:1],
                                    None, ALU.is_equal)
            ident_bf = cp.tile([128, 128], BF16)
            nc.vector.tensor_copy(ident_bf[:], ident[:])

            dstoffT_sb = cp.tile([128, NW * K], F32)
            nc.sync.dma_start(dstoffT_sb[:], dstoffT[:, :])
            dstoffT_bf = cp.tile([128, NW * K], BF16)
            nc.vector.tensor_copy(dstoffT_bf[:], dstoffT_sb[:])

            W1e_sb = cp.tile([128, F1 + 8], BF16)
            nc.sync.dma_start(W1e_sb[:], W1ext[:, :])
            W2a_sb = cp.tile([128, F2], BF16)
            nc.sync.dma_start(W2a_sb[:], W2a[:, :])
            W2b_sb = cp.tile([128, F2], BF16)
            nc.sync.dma_start(W2b_sb[:], W2b[:, :])
            vw2a_sb = cp.tile([128, 8], BF16)
            nc.sync.dma_start(vw2a_sb[:], vw2a[:, :])
            vw2b_sb = cp.tile([128, 8], BF16)
            nc.sync.dma_start(vw2b_sb[:], vw2b[:, :])

            ones = cp.tile([1, 128], F32)
            nc.vector.memset(ones[:], 1.0)
            epsb = cp.tile([128, 1], F32)
            nc.vector.memset(epsb[:], EPS_LN)
            zrow = cp.tile([128, 64], F32)
            nc.vector.memset(zrow[:], 0.0)
            for t_ in (ed1_lo, ed1_hi, ed2_lo, ed2_hi):
                nc.sync.dma_start(t_[0:1, :], zrow[0:1, :])

            bc = {}
            with tc.tile_pool(name="bcp", bufs=2, space="PSUM") as bps, \
                 tc.tile_pool(name="bcs", bufs=1) as bsb:
                for nm, d in [("b1", F1), ("b2", C2), ("gamma", C2),
                              ("beta", C2)]:
                    vsb = bsb.tile([1, d], F32, tag="vload")
                    nc.sync.dma_start(vsb[:], vecs[nm][:, :])
                    t = cp.tile([128, d], F32, tag=f"bc_{nm}")
                    ps = bps.tile([128, d], F32, tag="bcps")
                    nc.tensor.matmul(ps[:], ones[:], vsb[:], start=True,
                                     stop=True)
                    nc.vector.tensor_copy(t[:], ps[:])
                    bc[nm] = t

            # lo/hi split of a global window g (each CH0-chunk is within one
            # core's range; HALF is a core boundary)
            def ed_target(layer, g):
                lo = (g * WIN) < HALF
                tab = (ed1_lo if layer == 1 else ed2_lo) if lo else \
                      (ed1_hi if layer == 1 else ed2_hi)
                base = g * WIN + 1 - (0 if lo else HALF)
                return tab, base

            for _rep in range(REPEAT):
                # ================= Phase 0: full t1 table =================
                if "0" in PH:
                    with tc.tile_pool(name="p0", bufs=2) as p0, \
                         tc.tile_pool(name="p0ps", bufs=4, space="PSUM") as p0ps:
                        NWG = TOT // WIN  # 392 global windows
                        for g0 in range(0, NWG, CH0):
                            nw_c = CH0
                            xt = p0.tile([128, nw_c * 128], BF16, tag="xt")
                            nc.sync.dma_start(
                                xt[:], xT[:, g0 * WIN:(g0 + nw_c) * WIN])
                            pk = p0.tile([128, nw_c * R1], BF16, tag="pk")
                            edc = p0.tile([128, nw_c * 4], F32, tag="edc")
                            for i in range(nw_c):
                                ps = p0ps.tile([128, F1 + 8], F32, tag="ps")
                                nc.tensor.matmul(
                                    ps[:], xt[:, i * 128:(i + 1) * 128],
                                    W1e_sb[:], start=True, stop=True)
                                nc.scalar.copy(
                                    pk[:, i * R1:i * R1 + F1], ps[:, 0:F1])
                                nc.vector.tensor_copy(
                                    pk[:, i * R1 + F1:i * R1 + F1 + 8]
                                    .bitcast(F32), ps[:, F1:F1 + 4])
                                nc.vector.tensor_copy(
                                    edc[:, i * 4:(i + 1) * 4],
                                    ps[:, F1 + 4:F1 + 8])
                            nc.sync.dma_start(
                                t1[g0 * WIN:(g0 + nw_c) * WIN, :].rearrange(
                                    "(t p) e -> p t e", p=128),
                                pk[:].rearrange("p (t e) -> p t e", e=R1))
                            tab, base = ed_target(1, g0)
                            nc.sync.dma_start(
                                tab[base:base + nw_c * WIN, 0:4].rearrange(
                                    "(t p) e -> p t e", p=128),
                                edc[:].rearrange("p (t e) -> p t e", e=4))

                # ================= edge phases =================
                def edge_phase(layer):
                    F = F1 if layer == 1 else F2
                    C = C1 if layer == 1 else C2
                    R = R1 if layer == 1 else R2
                    GRP = GRP1 if layer == 1 else GRP2
                    tfull = t1 if layer == 1 else t2
                    exl, exh = (ed1_lo, ed1_hi) if layer == 1 else \
                               (ed2_lo, ed2_hi)
                    ixl, ixh = (idx_lo, idx_hi)
                    sfx = f"L{layer}"

                    with tc.tile_pool(name=f"edw{sfx}", bufs=1) as pw:
                        ed_win = pw.tile([128, NW * 4], BF16)
                        with tc.tile_pool(name=f"edg{sfx}", bufs=1) as pg:
                            gl = pg.tile([128, NW * 64], F32, tag="gl")
                            gh = pg.tile([128, NW * 64], F32, tag="gh")
                            eidl = pg.tile([128, SL // 16], I16, tag="eidl")
                            eidh = pg.tile([128, SL // 16], I16, tag="eidh")
                            nc.sync.dma_start(eidl[:], edidx_lo[:, :])
                            nc.sync.dma_start(eidh[:], edidx_hi[:, :])
                            nc.gpsimd.dma_gather(
                                gl[:].rearrange("p (t e) -> p t e", e=64),
                                exl[:, :], eidl[:], SL, SL, 64,
                                single_packet=False)
                            nc.gpsimd.dma_gather(
                                gh[:].rearrange("p (t e) -> p t e", e=64),
                                exh[:, :], eidh[:], SL, SL, 64,
                                single_packet=False)
                            nc.vector.tensor_tensor(
                                ed_win[:].rearrange("p (t e) -> p t e", e=4),
                                gl[:].rearrange("p (t e) -> p t e", e=64)
                                    [:, :, 0:4],
                                gh[:].rearrange("p (t e) -> p t e", e=64)
                                    [:, :, 0:4], ALU.add)

                        with tc.tile_pool(name=f"pe{sfx}", bufs=1) as pe, \
                             tc.tile_pool(name=f"pc{sfx}", bufs=2 if layer == 1 else 1) as pc, \
                             tc.tile_pool(name=f"pps{sfx}", bufs=2, space="PSUM") as pps, \
                             tc.tile_pool(name=f"ptps{sfx}", bufs=2, space="PSUM") as tps, \
                             tc.tile_pool(name=f"po{sfx}", bufs=2) as po, \
                             tc.tile_pool(name=f"pops{sfx}", bufs=2, space="PSUM") as ops:
                            for w0 in range(0, NW, GRP):
                                ng = min(GRP, NW - w0)
                                ilo = pc.tile([128, GRP * KL * 8], I16, tag="ilo")
                                nc.sync.dma_start(
                                    ilo[:, 0:ng * KL * 8],
                                    ixl[:, w0 * KL * 8:(w0 + ng) * KL * 8])
                                ihi = pc.tile([128, GRP * KH * 8], I16, tag="ihi")
                                nc.sync.dma_start(
                                    ihi[:, 0:ng * KH * 8],
                                    ixh[:, w0 * KH * 8:(w0 + ng) * KH * 8])
                                glo = pe.tile([128, GRP * KL * R], BF16, tag="glo")
                                ghi = pe.tile([128, GRP * KH * R], BF16, tag="ghi")
                                glov = glo[:].rearrange("p (t e) -> p t e", e=R)
                                ghiv = ghi[:].rearrange("p (t e) -> p t e", e=R)
                                nc.gpsimd.dma_gather(
                                    glov[:, 0:ng * KL, :], tfull[0:HALF, :],
                                    ilo[:, 0:ng * KL * 8], ng * KL * WIN,
                                    ng * KL * WIN, R, single_packet=False)
                                nc.gpsimd.dma_gather(
                                    ghiv[:, 0:ng * KH, :], tfull[HALF:TOT + 1, :],
                                    ihi[:, 0:ng * KH * 8], ng * KH * WIN,
                                    ng * KH * WIN, R, single_packet=False)

                                if layer == 1:
                                    oTa = po.tile([128, GRP * 128], BF16, tag="oTa")
                                    oTb = po.tile([128, GRP * 128], BF16, tag="oTb")
                                else:
                                    og = po.tile([128, GRP * C2], F32, tag="og")

                                for wg in range(ng):
                                    w = w0 + wg
                                    S_all = pc.tile([128, K * 128], BF16, tag="S")
                                    nc.vector.tensor_tensor(
                                        S_all[:].rearrange("p (k j) -> p k j", j=128),
                                        iotaK_bf[:].rearrange("p (k j) -> p k j", j=128),
                                        dstoffT_bf[:, w * K:(w + 1) * K]
                                            .unsqueeze(-1).broadcast_to([128, K, 128]),
                                        ALU.is_equal)
                                    psE = tps.tile([128, K * 4], F32, tag="psE")
                                    Tsb = pc.tile([128, K * 128], BF16, tag="Tsb")
                                    for b in range(K):
                                        psT = tps.tile([128, 128], BF16, tag="psT")
                                        nc.tensor.transpose(
                                            psT[:], S_all[:, b * 128:(b + 1) * 128],
                                            ident_bf[:])
                                        nc.scalar.copy(
                                            Tsb[:, b * 128:(b + 1) * 128], psT[:])
                                        nc.tensor.matmul(
                                            psE[:, b * 4:(b + 1) * 4],
                                            Tsb[:, b * 128:(b + 1) * 128],
                                            ed_win[:, w * 4:(w + 1) * 4],
                                            start=True, stop=True)
                                    e_all = pc.tile([128, K * 4], F32, tag="e_all")
                                    ev = e_all[:].rearrange("p (k h) -> p k h", h=4)
                                    psEv = psE[:].rearrange("p (k h) -> p k h", h=4)
                                    nc.vector.tensor_tensor(
                                        ev[:, 0:KL, :],
                                        glov[:, wg * KL:(wg + 1) * KL, F:F + 8]
                                            .bitcast(F32),
                                        psEv[:, 0:KL, :], ALU.add)
                                    nc.vector.tensor_tensor(
                                        ev[:, KL:K, :],
                                        ghiv[:, wg * KH:(wg + 1) * KH, F:F + 8]
                                            .bitcast(F32),
                                        psEv[:, KL:K, :], ALU.add)
                                    e_sc = pc.tile([128, K * 4], F32, tag="e_sc")
                                    nc.vector.tensor_scalar(
                                        e_sc[:], e_all[:], NEG_SLOPE, None, ALU.mult)
                                    nc.vector.tensor_tensor(
                                        e_all[:], e_all[:], e_sc[:], ALU.max)
                                    w_bf = pc.tile([128, K * 4], BF16, tag="w_bf")
                                    nc.scalar.activation(w_bf[:], e_all[:], AF.Exp)
                                    wv = w_bf[:].rearrange("p (k h) -> p k h", h=4)

                                    RC = F + 4
                                    rhs = pc.tile([128, K * RC], BF16, tag="rhs")
                                    rv = rhs[:].rearrange("p (k r) -> p k r", r=RC)
                                    nc.vector.tensor_tensor(
                                        rv[:, 0:KL, 0:F].rearrange(
                                            "p k (h c) -> p k h c", c=C),
                                        glov[:, wg * KL:(wg + 1) * KL, 0:F]
                                            .rearrange("p k (h c) -> p k h c", c=C),
                                        wv[:, 0:KL, :].unsqueeze(-1)
                                            .broadcast_to([128, KL, 4, C]),
                                        ALU.mult)
                                    nc.vector.tensor_tensor(
                                        rv[:, KL:K, 0:F].rearrange(
                                            "p k (h c) -> p k h c", c=C),
                                        ghiv[:, wg * KH:(wg + 1) * KH, 0:F]
                                            .rearrange("p k (h c) -> p k h c", c=C),
                                        wv[:, KL:K, :].unsqueeze(-1)
                                            .broadcast_to([128, KH, 4, C]),
                                        ALU.mult)
                                    nc.vector.tensor_copy(rv[:, :, F:F + 4], wv)

                                    if layer == 1:
                                        psA = pps.tile([128, F1 + 4], F32, tag="psA")
                                        for b in range(K):
                                            nc.tensor.matmul(
                                                psA[:], S_all[:, b * 128:(b + 1) * 128],
                                                rv[:, b, :], start=(b == 0),
                                                stop=(b == K - 1))
                                        # epilogue 1
                                        den = po.tile([128, 4], F32, tag="den")
                                        nc.vector.tensor_scalar(
                                            den[:], psA[:, F1:F1 + 4], 1e-30,
                                            None, ALU.max)
                                        rec = po.tile([128, 4], F32, tag="rec")
                                        nc.vector.reciprocal(rec[:], den[:])
                                        o1f = po.tile([128, F1], F32, tag="o1f")
                                        nc.vector.tensor_tensor(
                                            o1f[:].rearrange("p (h c) -> p h c", c=C1),
                                            psA[:, 0:F1].rearrange(
                                                "p (h c) -> p h c", c=C1),
                                            rec[:].unsqueeze(-1)
                                                .broadcast_to([128, 4, C1]),
                                            ALU.mult)
                                        nc.vector.tensor_tensor(
                                            o1f[:], o1f[:], bc["b1"][:], ALU.add)
                                        nc.scalar.activation(o1f[:], o1f[:], AF.Relu)
                                        psto = ops.tile([128, 128], F32, tag="psto")
                                        nc.tensor.transpose(
                                            psto[:], o1f[:, 0:128], ident[:])
                                        nc.scalar.copy(
                                            oTa[:, wg * 128:(wg + 1) * 128], psto[:])
                                        psto2 = ops.tile([128, 128], F32, tag="psto")
                                        nc.tensor.transpose(
                                            psto2[:], o1f[:, 128:256], ident[:])
                                        nc.scalar.copy(
                                            oTb[:, wg * 128:(wg + 1) * 128], psto2[:])
                                    else:
                                        psA2 = pps.tile([128, F2], F32, tag="psA2")
                                        psD = tps.tile([128, 4], F32, tag="psD")
                                        for b in range(K):
                                            S_b = S_all[:, b * 128:(b + 1) * 128]
                                            nc.tensor.matmul(
                                                psA2[:], S_b, rv[:, b, 0:F2],
                                                start=(b == 0), stop=(b == K - 1))
                                            nc.tensor.matmul(
                                                psD[:], S_b, rv[:, b, F2:F2 + 4],
                                                start=(b == 0), stop=(b == K - 1))
                                        # epilogue 2
                                        den = po.tile([128, 4], F32, tag="den2")
                                        nc.vector.tensor_scalar(
                                            den[:], psD[:], 1e-30, None, ALU.max)
                                        rec = po.tile([128, 4], F32, tag="rec2")
                                        nc.vector.reciprocal(rec[:], den[:])
                                        tmp = po.tile([128, F2], F32, tag="tmp")
                                        nc.vector.tensor_tensor(
                                            tmp[:].rearrange("p (h c) -> p h c", c=C2),
                                            psA2[:].rearrange("p (h c) -> p h c", c=C2),
                                            rec[:].unsqueeze(-1)
                                                .broadcast_to([128, 4, C2]),
                                            ALU.mult)
                                        o2 = po.tile([128, C2], F32, tag="o2")
                                        nc.vector.reduce_sum(
                                            o2[:], tmp[:].rearrange(
                                                "p (h c) -> p c h", c=C2),
                                            axis=mybir.AxisListType.X)
                                        nc.vector.tensor_scalar(
                                            o2[:], o2[:], 1.0 / H, None, ALU.mult)
                                        nc.vector.tensor_tensor(
                                            o2[:], o2[:], bc["b2"][:], ALU.add)
                                        mu = po.tile([128, 1], F32, tag="mu")
                                        nc.vector.reduce_sum(
                                            mu[:], o2[:], axis=mybir.AxisListType.X)
                                        nc.vector.tensor_scalar(
                                            mu[:], mu[:], 1.0 / C2, None, ALU.mult)
                                        xc = po.tile([128, C2], F32, tag="xc")
                                        nc.vector.tensor_scalar(
                                            xc[:], o2[:], mu[:], None, ALU.subtract)
                                        sq = po.tile([128, C2], F32, tag="sq")
                                        ssq = po.tile([128, 1], F32, tag="ssq")
                                        nc.scalar.activation(
                                            sq[:], xc[:], AF.Square, accum_out=ssq[:])
                                        sdev = po.tile([128, 1], F32, tag="sdev")
                                        nc.scalar.activation(
                                            sdev[:], ssq[:], AF.Sqrt,
                                            scale=1.0 / C2, bias=epsb[:, 0:1])
                                        rstd = po.tile([128, 1], F32, tag="rstd")
                                        nc.vector.reciprocal(rstd[:], sdev[:])
                                        xn = po.tile([128, C2], F32, tag="xn")
                                        nc.vector.tensor_scalar(
                                            xn[:], xc[:], rstd[:], None, ALU.mult)
                                        nc.vector.tensor_tensor(
                                            xn[:], xn[:], bc["gamma"][:], ALU.mult)
                                        nc.vector.tensor_tensor(
                                            og[:, wg * C2:(wg + 1) * C2], xn[:],
                                            bc["beta"][:], ALU.add)

                                if layer == 1:
                                    nc.sync.dma_start(
                                        o1T_shard[0:128, w0 * 128:(w0 + ng) * 128],
                                        oTa[:, 0:ng * 128])
                                    nc.sync.dma_start(
                                        o1T_shard[128:256, w0 * 128:(w0 + ng) * 128],
                                        oTb[:, 0:ng * 128])
                                else:
                                    nc.sync.dma_start(
                                        out[w0 * WIN:(w0 + ng) * WIN, :].rearrange(
                                            "(t p) e -> p t e", p=128),
                                        og[:, 0:ng * C2].rearrange(
                                            "p (t e) -> p t e", e=C2))

                if "1" in PH:
                    edge_phase(1)
                if "A" in PH:
                    nc.gpsimd.collective_compute(
                        "AllGather", ALU.bypass, replica_groups=RG,
                        ins=[o1T_shard[:, :]], outs=[o1T_full[:, :]])
                # ============ interlayer: full t2 from o1T_full ============
                if "I" in PH:
                    with tc.tile_pool(name="il", bufs=2) as il, \
                         tc.tile_pool(name="ilps", bufs=4, space="PSUM") as ilps:
                        for c in range(CORES):
                            for w0 in range(0, NW, CH0):
                                nw_c = CH0
                                oa = il.tile([128, nw_c * 128], BF16, tag="oa")
                                ob = il.tile([128, nw_c * 128], BF16, tag="ob")
                                cols = slice(w0 * 128, (w0 + nw_c) * 128)
                                nc.sync.dma_start(
                                    oa[:], o1T_full[c * 256:c * 256 + 128, cols])
                                nc.sync.dma_start(
                                    ob[:], o1T_full[c * 256 + 128:c * 256 + 256, cols])
                                pk2 = il.tile([128, nw_c * R2], BF16, tag="pk2")
                                ed2c = il.tile([128, nw_c * 4], F32, tag="ed2c")
                                for i in range(nw_c):
                                    psH = ilps.tile([128, F2], F32, tag="psH")
                                    nc.tensor.matmul(
                                        psH[:], oa[:, i * 128:(i + 1) * 128],
                                        W2a_sb[:], start=True, stop=False)
                                    nc.tensor.matmul(
                                        psH[:], ob[:, i * 128:(i + 1) * 128],
                                        W2b_sb[:], start=False, stop=True)
                                    psV = ilps.tile([128, 8], F32, tag="psV")
                                    nc.tensor.matmul(
                                        psV[:], oa[:, i * 128:(i + 1) * 128],
                                        vw2a_sb[:], start=True, stop=False)
                                    nc.tensor.matmul(
                                        psV[:], ob[:, i * 128:(i + 1) * 128],
                                        vw2b_sb[:], start=False, stop=True)
                                    nc.scalar.copy(
                                        pk2[:, i * R2:i * R2 + F2], psH[:])
                                    nc.vector.tensor_copy(
                                        pk2[:, i * R2 + F2:i * R2 + F2 + 8]
                                        .bitcast(F32), psV[:, 0:4])
                                    nc.vector.tensor_copy(
                                        ed2c[:, i * 4:(i + 1) * 4], psV[:, 4:8])
                                g0 = c * NW + w0
                                nc.sync.dma_start(
                                    t2[g0 * WIN:(g0 + nw_c) * WIN, :].rearrange(
                                        "(t p) e -> p t e", p=128),
                                    pk2[:].rearrange("p (t e) -> p t e", e=R2))
                                tab, base = ed_target(2, g0)
                                nc.sync.dma_start(
                                    tab[base:base + nw_c * WIN, 0:4].rearrange(
                                        "(t p) e -> p t e", p=128),
                                    ed2c[:].rearrange("p (t e) -> p t e", e=4))
                if "2" in PH:
                    edge_phase(2)

    nc.finalize()
    return nc


# --------------------------------------------------------------------------
_CACHE = {}


def run(inputs, trace=False):
    cfg, in_maps, slot_global = prep(**inputs)
    ckey = (cfg.key, os.environ.get("GAT_PHASES"), os.environ.get("GAT_REPEAT"),
            os.environ.get("GAT_GRP1"), os.environ.get("GAT_GRP2"))
    nc = _CACHE.get(ckey)
    if nc is None:
        nc = build(cfg)
        _CACHE[ckey] = nc
    res = run_bass_kernel_spmd(nc, in_maps, core_ids=list(range(CORES)),
                               trace=trace)
    full = np.concatenate([res.results[c]["out"] for c in range(CORES)],
                          axis=0)
    return full[slot_global], res


def kernel(**inputs):
    """Full unsharded inputs -> full [N, 128] output (runs on 8 NeuronCores)."""
    out, _ = run(inputs)
    return out
